# revision 7
# baseline (speedup 1.0000x reference)
"""Trainium2 Bass kernel for nn_CrossChannelAttention.

Reference computation (per batch b, pixel p, with C=128 channels, NUMS=16
groups of HEADS=8 channels, OUT=256):
    fm[g,p]  = relu(sum_h W1[g,h] * x[8g+h, p] + b1[g])          # [16, P]
    feat[(g,d), p] = fm[g,p] * x[d,p]                            # [2048, P]
    out[o,p] = sum_c W2[o,c] * feat[c,p] + b2[o]                 # [256, P]

Strategy: data-parallel over batch B=8 across the 8 NeuronCores (one image
per core, params replicated).  Per core the PE-bound floor is 256 bf16
matmuls [K=128,M=128,N=512] ~= 57us; everything else must hide under it.

v3 redesign vs the 90.8us v2 (trace analysis: sync engine 44us busy just
issuing ~76 DMA triggers at ~0.6us each; 59 per-(g,k) [128,1024] rep
broadcasts = 15MB of 2KB-packet SBUF writes saturating the 16 DMA engines;
~10us framework epilogue scaling with tile/DMA counts; HAM clock reaches
2.4GHz only after ~12us of sustained PE activity):
  - Phase structure: two 2048-pixel phases, each using all 8 PSUM banks
    (4 pixel blocks x 2 output-channel halves), accumulating all 16 groups.
  - One full-width [128,2048] rep broadcast DMA per (g, phase) instead of
    per-(g,k) units: 30 rep triggers instead of 59, split across the sync
    and scalar HWDGE rings.
  - Phase B reps prefetch during phase A (DMA is otherwise idle then), so
    phase B never waits on DMA.
  - g=0 reps via gpsimd partition_broadcast straight from the fm SBUF tile
    (partition 0 is the only legal gpsimd source = row g=0), in [128,512]
    quarters for phase A to bootstrap the pipeline with no DMA hops.
  - fm psum banks chosen so fmB's matmuls interleave between the g=0 main
    matmuls without colliding with main accumulators; warmup matmuls (DVE-
    memset scratch) run from ~7us to ramp the HAM clock gate.
  - Drains chase the PE per bank (scalar=oc0, vector=oc1); stores are 4x
    [128,1024] bf16 per phase split across both rings.
Accuracy: bf16 matmuls with fp32 PSUM accumulation; rel err ~4e-3.
"""

import numpy as np
import ml_dtypes

import concourse.bacc as bacc
import concourse.tile as tile
from concourse import mybir
from concourse.bass_utils import run_bass_kernel_spmd

F32 = mybir.dt.float32
BF16 = mybir.dt.bfloat16

B, C, H, W = 8, 128, 64, 64
NUMS, HEADS, OUT = 16, 8, 256
P = H * W          # 4096 pixels per image
PB = 512           # pixel block (one PSUM bank of fp32)
PH = 2048          # phase width (4 pixel blocks; all 8 PSUM banks)
NPH = P // PH      # 2 phases
N_CORES = 8

_CACHE = {}


def _build():
    nc = bacc.Bacc("TRN2", target_bir_lowering=False, debug=False,
                   num_devices=N_CORES)

    x_d = nc.dram_tensor("x", [C, P], BF16, kind="ExternalInput")
    # Small params padded to >=512B per partition row: shorter DMA rows
    # fall into SDMA read-modify-write with ~800ns/descriptor.
    w1s_d = nc.dram_tensor("w1s", [C, 256], BF16, kind="ExternalInput")
    w2t_d = nc.dram_tensor("w2t", [C, NUMS * OUT], BF16, kind="ExternalInput")
    b1_d = nc.dram_tensor("b1c", [NUMS, 128], F32, kind="ExternalInput")
    b2_d = nc.dram_tensor("b2c", [C, 128], F32, kind="ExternalInput")
    # bf16 output halves the 4MB store traffic; ~0.2% extra error vs the
    # 2e-2 budget (host converts back to fp32)
    out_d = nc.dram_tensor("out", [OUT, P], BF16, kind="ExternalOutput")

    relu = mybir.ActivationFunctionType.Relu
    ident = mybir.ActivationFunctionType.Identity
    mult = mybir.AluOpType.mult

    with tile.TileContext(nc) as tc:
        with (
            tc.tile_pool(name="const", bufs=1) as cpool,
            tc.tile_pool(name="repA", bufs=1) as repAp,
            tc.tile_pool(name="repB", bufs=1) as repBp,
            tc.tile_pool(name="ft", bufs=1) as ftp,
            tc.tile_pool(name="osb", bufs=1) as osbp,
            tc.tile_pool(name="ps", bufs=1, space="PSUM") as ps,
            tc.tile_pool(name="dr", bufs=1, space="DRAM") as drp,
        ):
            # ---- constants / inputs ----
            scratch = cpool.tile([C, C + PB], BF16)
            nc.vector.memset(scratch[:], 0.0)

            w1s_t = cpool.tile([C, 256], BF16)
            b1_t = cpool.tile([NUMS, 128], F32)
            b2_t = cpool.tile([C, 128], F32)
            xA = cpool.tile([C, PH], BF16, name="xA")
            xB = cpool.tile([C, PH], BF16, name="xB")
            w2c = [cpool.tile([C, PH], BF16, name=f"w2c{j}") for j in range(2)]

            # sync ring: pure loads in need-order
            nc.sync.dma_start(w1s_t[:], w1s_d[:])
            nc.sync.dma_start(xA[:], x_d[:, 0:PH])
            nc.sync.dma_start(w2c[0][:], w2t_d[:, 0:PH])
            nc.sync.dma_start(xB[:], x_d[:, PH:P])
            nc.sync.dma_start(w2c[1][:], w2t_d[:, PH:2 * PH])
            # scalar ring: biases
            nc.scalar.dma_start(b1_t[:], b1_d[:])
            nc.scalar.dma_start(b2_t[:], b2_d[:])

            # ---- PSUM tiles: 8 banks, tag-per-bank, serial reuse ----
            def psum(pb, oc, nm, parts=C):
                return ps.tile([parts, PB], F32, tag=f"pso_{pb}_{oc}",
                               name=nm)

            # warmup matmuls ramp the HAM clock gate from ~7us
            ps_w = psum(0, 0, "ps_warm")
            for i in range(3):
                nc.tensor.matmul(ps_w[:], scratch[:, 0:C],
                                 scratch[:, C:C + PB], start=True, stop=True)

            # ---- fm phase A: 4 matmuls [16,512] + relus, gpsimd g=0 reps ----
            fmhA = cpool.tile([NUMS, PH], BF16, name="fmhA")
            fmhB = cpool.tile([NUMS, PH], BF16, name="fmhB")
            fm_drA = drp.tile([NUMS, PH], BF16, name="fm_drA")
            fm_drB = drp.tile([NUMS, PH], BF16, name="fm_drB")
            fm_psA_tags = [(1, 0), (1, 1), (2, 0), (2, 1)]
            for i in range(4):
                qx = slice(i * PB, (i + 1) * PB)
                pf = psum(*fm_psA_tags[i], nm=f"psfmA{i}", parts=NUMS)
                nc.tensor.matmul(pf[:], w1s_t[:, 0:NUMS], xA[:, qx],
                                 start=True, stop=True)
                nc.scalar.activation(fmhA[:, qx], pf[:], relu,
                                     bias=b1_t[:, 0:1])
                if i == 1:
                    nc.scalar.dma_start(fm_drA[:, 0:1024],
                                        fmhA[0:NUMS, 0:1024])
                if i == 3:
                    nc.scalar.dma_start(fm_drA[:, 1024:PH],
                                        fmhA[0:NUMS, 1024:PH])
            # one more warmup while rep0 quarter 0 is in flight
            nc.tensor.matmul(ps_w[:], scratch[:, 0:C], scratch[:, C:C + PB],
                             start=True, stop=True)
            # phase-B fm psum tiles created NOW so the pso_3_* tag rotation
            # puts them BEFORE phase A's main accumulators (their matmuls
            # interleave into mains g=0, well before mains touch pb3)
            fm_psB_tags = [(3, 0), (3, 1), (3, 0), (3, 1)]
            pfB = [psum(*fm_psB_tags[i], nm=f"psfmB{i}", parts=NUMS)
                   for i in range(4)]

            # g=0 phase-A reps via gpsimd in [128,512] quarters (partition 0
            # of fmhA is row g=0 - the only legal gpsimd source); no DMA hop
            rep0q = []
            for i in range(4):
                qx = slice(i * PB, (i + 1) * PB)
                r = repAp.tile([C, PB], BF16, name=f"rep0q{i}")
                nc.gpsimd.partition_broadcast(r[:], fmhA[0:1, qx])
                rep0q.append(r)

            # ---- rep DMA broadcasts, phase A ----
            # ALL phase-A rep DMAs are emitted BEFORE their ft consumers
            # (trace order defines the dependency direction in Tile).
            # g=1 in two [128,1024] halves (earlier availability); g>=2 full
            # [128,2048].  Even g on the (idle) sync ring, odd g on scalar.
            rep1h = []
            for h in range(2):
                r = repAp.tile([C, 1024], BF16, name=f"rep1h{h}")
                hx = slice(h * 1024, (h + 1) * 1024)
                nc.sync.dma_start(
                    r[:], fm_drA[1:2, hx].broadcast_to((C, 1024)))
                rep1h.append(r)
            repA = {}
            for g in range(2, NUMS):
                repA[g] = repAp.tile([C, PH], BF16, tag="repfA", bufs=8,
                                     name=f"repA{g}")

            def emit_repA(g):
                eng = nc.sync if (g % 2 == 0) else nc.scalar
                eng.dma_start(repA[g][:],
                              fm_drA[g:g + 1, :].broadcast_to((C, PH)))

            for g in range(2, NUMS, 2):
                emit_repA(g)
            for g in range(3, NUMS, 2):
                emit_repA(g)

            # ---- feat producers (DVE), phase A g=0,1 ----
            ftA = {}
            for i in range(4):
                qx = slice(i * PB, (i + 1) * PB)
                f = ftp.tile([C, PB], BF16, name=f"ft0q{i}")
                nc.vector.tensor_tensor(f[:], xA[:, qx], rep0q[i][:], op=mult)
                ftA[(0, i)] = f

            def rhsA(g, pb):
                if g == 0:
                    return ftA[(0, pb)][:]
                if g == 1:
                    h, r = divmod(pb, 2)
                    return ftA[(1, h)][:, r * PB:(r + 1) * PB]
                return ftA[g][:, pb * PB:(pb + 1) * PB]

            def w2blk(g, oc):
                j, r = divmod(g, 8)
                cx = slice((r * 2 + oc) * C, (r * 2 + oc + 1) * C)
                return w2c[j][:, cx]

            # ---- main matmuls phase A, with fmB interleaved into g=0 ----
            psoA = {(pb, oc): psum(pb, oc, f"psoA{pb}_{oc}")
                    for pb in range(4) for oc in range(2)}

            def emit_fmB(i):
                # relu on DVE (tensor_scalar add+max) so the scalar ring's
                # rep-trigger backlog can't delay it and stall the PE
                qx = slice(i * PB, (i + 1) * PB)
                nc.tensor.matmul(pfB[i][:], w1s_t[:, 0:NUMS], xB[:, qx],
                                 start=True, stop=True)
                nc.vector.tensor_scalar(fmhB[:, qx], pfB[i][:],
                                        b1_t[:, 0:1], 0.0,
                                        op0=mybir.AluOpType.add,
                                        op1=mybir.AluOpType.max)

            for g in range(NUMS):
                for pb in range(4):
                    for oc in range(2):
                        nc.tensor.matmul(psoA[(pb, oc)][:], w2blk(g, oc),
                                         rhsA(g, pb),
                                         start=(g == 0), stop=(g == NUMS - 1))
                    if g == 0 and pb == 0:
                        emit_fmB(0)
                        emit_fmB(1)
                    if g == 0 and pb == 1:
                        emit_fmB(2)
                        emit_fmB(3)
                if g == 0:
                    # DVE stream: ft1 halves right after the fmB relus
                    for h in range(2):
                        hx = slice(h * 1024, (h + 1) * 1024)
                        f = ftp.tile([C, 1024], BF16, name=f"ft1h{h}")
                        nc.vector.tensor_tensor(f[:], xA[:, hx],
                                                rep1h[h][:], op=mult)
                        ftA[(1, h)] = f
                    # phase-B g=0 rep via gpsimd (single [128,2048] op) and
                    # fm_drB writes on the scalar ring
                    rep0B = repBp.tile([C, PH], BF16, name="rep0B")
                    nc.gpsimd.partition_broadcast(rep0B[:], fmhB[0:1, :])
                    nc.scalar.dma_start(fm_drB[:, 0:1024],
                                        fmhB[0:NUMS, 0:1024])
                    nc.scalar.dma_start(fm_drB[:, 1024:PH],
                                        fmhB[0:NUMS, 1024:PH])
                    # remaining phase-A feat producers (reps already issued)
                    for gg in range(2, NUMS):
                        f = ftp.tile([C, PH], BF16, tag="ftfA", bufs=5,
                                     name=f"ftA{gg}")
                        nc.vector.tensor_tensor(f[:], xA[:], repA[gg][:],
                                                op=mult)
                        ftA[gg] = f
                if g == 1:
                    repB = {}
                    for gb in range(1, NUMS):
                        r = repBp.tile([C, PH], BF16, tag="repfB", bufs=15,
                                       name=f"repB{gb}")
                        eng = nc.sync if (gb % 2 == 0) else nc.scalar
                        eng.dma_start(
                            r[:], fm_drB[gb:gb + 1, :].broadcast_to((C, PH)))
                        repB[gb] = r

            # ---- feat producers phase B (start of DVE stream tail) ----
            ftB = {}
            f = ftp.tile([C, PH], BF16, name="ftB0")
            nc.vector.tensor_tensor(f[:], xB[:], rep0B[:], op=mult)
            ftB[0] = f
            for g in range(1, 3):
                f = ftp.tile([C, PH], BF16, tag="ftfB", bufs=6,
                             name=f"ftB{g}")
                nc.vector.tensor_tensor(f[:], xB[:], repB[g][:], op=mult)
                ftB[g] = f

            # ---- drains + stores phase A (chase the PE bank order) ----
            osbA = {(pp, oc): osbp.tile([C, 1024], BF16, tag="osb", bufs=4,
                                        name=f"osbA{pp}_{oc}")
                    for pp in range(2) for oc in range(2)}
            for pb in range(4):
                pp, r = divmod(pb, 2)
                sx = slice(r * PB, (r + 1) * PB)
                nc.scalar.activation(osbA[(pp, 0)][:, sx],
                                     psoA[(pb, 0)][:], ident,
                                     bias=b2_t[:, 0:1])
                nc.vector.tensor_scalar_add(osbA[(pp, 1)][:, sx],
                                            psoA[(pb, 1)][:], b2_t[:, 1:2])
            for pp in range(2):
                px = slice(pp * 1024, (pp + 1) * 1024)
                nc.sync.dma_start(out_d[0:C, px], osbA[(pp, 0)][:])
                nc.scalar.dma_start(out_d[C:OUT, px], osbA[(pp, 1)][:])

            # remaining phase-B feat producers
            for g in range(3, NUMS):
                f = ftp.tile([C, PH], BF16, tag="ftfB", bufs=6,
                             name=f"ftB{g}")
                nc.vector.tensor_tensor(f[:], xB[:], repB[g][:], op=mult)
                ftB[g] = f

            # ---- main matmuls phase B ----
            psoB = {(pb, oc): psum(pb, oc, f"psoB{pb}_{oc}")
                    for pb in range(4) for oc in range(2)}
            for g in range(NUMS):
                for pb in range(4):
                    rhs = ftB[g][:, pb * PB:(pb + 1) * PB]
                    for oc in range(2):
                        nc.tensor.matmul(psoB[(pb, oc)][:], w2blk(g, oc),
                                         rhs,
                                         start=(g == 0), stop=(g == NUMS - 1))

            # ---- drains + stores phase B ----
            osbB = {(pp, oc): osbp.tile([C, 1024], BF16, tag="osb", bufs=4,
                                        name=f"osbB{pp}_{oc}")
                    for pp in range(2) for oc in range(2)}
            for pb in range(4):
                pp, r = divmod(pb, 2)
                sx = slice(r * PB, (r + 1) * PB)
                nc.scalar.activation(osbB[(pp, 0)][:, sx],
                                     psoB[(pb, 0)][:], ident,
                                     bias=b2_t[:, 0:1])
                nc.vector.tensor_scalar_add(osbB[(pp, 1)][:, sx],
                                            psoB[(pb, 1)][:], b2_t[:, 1:2])
            for pp in range(2):
                px = slice(PH + pp * 1024, PH + (pp + 1) * 1024)
                nc.sync.dma_start(out_d[0:C, px], osbB[(pp, 0)][:])
                nc.scalar.dma_start(out_d[C:OUT, px], osbB[(pp, 1)][:])

    nc.compile()
    return nc


def _prep_params(W1, b1, W2, b2):
    bf = ml_dtypes.bfloat16
    # w1s[c, g] = W1[g, c - 8g] for 8g <= c < 8(g+1), else 0; padded to
    # 256 cols (512B DMA rows)
    w1s = np.zeros((C, 256), dtype=bf)
    for g in range(NUMS):
        w1s[g * HEADS:(g + 1) * HEADS, g] = W1[g].astype(bf)
    # w2t[k, (g*2+oc)*128 + m] = W2[oc*128 + m, g*128 + k]
    w2t = (
        np.asarray(W2, dtype=np.float32)
        .reshape(2, C, NUMS, C)          # [oc, m, g, k]
        .transpose(3, 2, 0, 1)           # [k, g, oc, m]
        .reshape(C, NUMS * OUT)
        .astype(bf)
    )
    b1c = np.zeros((NUMS, 128), dtype=np.float32)
    b1c[:, 0] = np.asarray(b1, dtype=np.float32)
    b2c = np.zeros((C, 128), dtype=np.float32)
    b2c[:, 0:2] = np.asarray(b2, dtype=np.float32).reshape(2, C).T
    return w1s, w2t, b1c, b2c


def kernel(x, W1, b1, W2, b2, _trace=False, _trace_kwargs=None):
    if "nc" not in _CACHE:
        _CACHE["nc"] = _build()
    nc = _CACHE["nc"]

    w1s, w2t, b1c, b2c = _prep_params(W1, b1, W2, b2)
    xs = np.ascontiguousarray(
        np.asarray(x, dtype=np.float32).reshape(B, C, P).astype(ml_dtypes.bfloat16))
    in_maps = [
        {"x": xs[b_], "w1s": w1s, "w2t": w2t, "b1c": b1c, "b2c": b2c}
        for b_ in range(N_CORES)
    ]
    kwargs = {}
    if _trace:
        kwargs["trace"] = True
        kwargs.update(_trace_kwargs or {})
    res = run_bass_kernel_spmd(nc, in_maps, core_ids=list(range(N_CORES)),
                               **kwargs)
    out = np.stack([np.asarray(res.results[b_]["out"], dtype=np.float32)
                    for b_ in range(N_CORES)])
    out = out.reshape(B, OUT, H, W)
    if _trace:
        _CACHE["last_result"] = res
    return out
